# revision 4
# baseline (speedup 1.0000x reference)
"""GCNN (spektral ECCConv x2 + attn-sum-pool + dense head) Trainium2 kernel.

Strategy
--------
The ECC "kernel network" is a single linear layer, so the per-edge kernel
tensor [B,N,N,Fo*F] never needs to exist.  Folding emb_w into kn_w gives
W16[s,o,f] (s over the 16 raw edge channels + 1 bias channel) and

    out[b,i,o] = sum_{j,s} a[b,i,j]*e[b,i,j,s] * P16[b,j,s,o] + (h @ root)[b,i,o]
    P16[b,j,s,o] = sum_f W16[s,o,f] * h[b,j,f]

Everything is kept feature-major ("transposed", [F, node]) on chip:
  - aT and the 16 e-slices are transposed once on the TensorEngine
    (AE_T[j, s*128+i] = a[b,i,j]*e[b,i,j,s], reused by both layers)
  - per layer: one [32,128]x[32,544] matmul produces P16, then 17
    accumulating [128,32]x[128,128] matmuls + one root matmul produce
    the layer output directly in [Fo, node] layout.
Distribution: data-parallel over batch (B=4) on cores 0-3; cores 4-7
process duplicate data and are ignored.

Shapes are hardcoded for B=4, N=128, F0=32, S=16, E8=8, C=[32,32].
"""

import numpy as np

import concourse.bass as bass
import concourse.mybir as mybir
import concourse.tile as tile
from concourse import bacc
from concourse.bass_utils import run_bass_kernel_spmd
from concourse.masks import make_identity

FP = mybir.dt.float32
AL = mybir.AluOpType
AF = mybir.ActivationFunctionType

N = 128       # nodes
F = 32        # feature dim of both ECC layers (F0 = C0 = C1 = 32)
S = 16        # raw edge feature dim
SB = S + 1    # + kernel-network bias channel
D = 1024      # dense head width
DC = D // N   # dense head column chunks (8)
EPS = 1e-3    # Keras BatchNormalization epsilon
ALPHA = 0.01  # LeakyReLU
B = 4
NCORES = 8

_NC_CACHE = None


def _build_nc():
    nc = bacc.Bacc("TRN2", target_bir_lowering=False, debug=False)

    e_b = nc.dram_tensor("e_b", [N, N * S], FP, kind="ExternalInput")
    a_b = nc.dram_tensor("a_b", [N, N], FP, kind="ExternalInput")
    x_b = nc.dram_tensor("x_b", [N, F], FP, kind="ExternalInput")
    w16_0 = nc.dram_tensor("w16_0", [F, SB * F], FP, kind="ExternalInput")
    w16_1 = nc.dram_tensor("w16_1", [F, SB * F], FP, kind="ExternalInput")
    root_0 = nc.dram_tensor("root_0", [F, F], FP, kind="ExternalInput")
    root_1 = nc.dram_tensor("root_1", [F, F], FP, kind="ExternalInput")
    bias_0 = nc.dram_tensor("bias_0", [F, 1], FP, kind="ExternalInput")
    bias_1 = nc.dram_tensor("bias_1", [F, 1], FP, kind="ExternalInput")
    bn0s = nc.dram_tensor("bn0s", [F, 1], FP, kind="ExternalInput")
    bn0h = nc.dram_tensor("bn0h", [F, 1], FP, kind="ExternalInput")
    bn1s = nc.dram_tensor("bn1s", [F, 1], FP, kind="ExternalInput")
    bn1h = nc.dram_tensor("bn1h", [F, 1], FP, kind="ExternalInput")
    attn_k = nc.dram_tensor("attn_k", [F, 1], FP, kind="ExternalInput")
    dense_w = nc.dram_tensor("dense_w", [F, D], FP, kind="ExternalInput")
    dense_bc = nc.dram_tensor("dense_bc", [N, DC], FP, kind="ExternalInput")
    wf = nc.dram_tensor("wf", [N, DC], FP, kind="ExternalInput")
    cf = nc.dram_tensor("cf", [1, 1], FP, kind="ExternalInput")
    y_out = nc.dram_tensor("y", [1, 1], FP, kind="ExternalOutput")

    with tile.TileContext(nc) as tc:
        with (
            tc.tile_pool(name="big", bufs=1) as big,
            tc.tile_pool(name="consts", bufs=1) as consts,
            tc.tile_pool(name="work", bufs=2) as work,
            tc.tile_pool(name="ps_tr", bufs=2, space="PSUM") as ps_tr,
            tc.tile_pool(name="ps_p16", bufs=2, space="PSUM") as ps_p16p,
            tc.tile_pool(name="ps_misc", bufs=1, space="PSUM") as ps_misc,
        ):
            # ---------------- loads ----------------
            e_sb = big.tile([N, N * S], FP)
            nc.sync.dma_start(out=e_sb, in_=e_b[:, :])
            a_sb = consts.tile([N, N], FP)
            nc.sync.dma_start(out=a_sb, in_=a_b[:, :])
            x_sb = consts.tile([N, F], FP)
            nc.sync.dma_start(out=x_sb, in_=x_b[:, :])

            w16_sb = [consts.tile([F, SB * F], FP, name=f"w16sb{i}") for i in range(2)]
            nc.sync.dma_start(out=w16_sb[0], in_=w16_0[:, :])
            nc.sync.dma_start(out=w16_sb[1], in_=w16_1[:, :])
            root_sb = [consts.tile([F, F], FP, name=f"rootsb{i}") for i in range(2)]
            nc.sync.dma_start(out=root_sb[0], in_=root_0[:, :])
            nc.sync.dma_start(out=root_sb[1], in_=root_1[:, :])
            bias_sb = [consts.tile([F, 1], FP, name=f"biassb{i}") for i in range(2)]
            nc.sync.dma_start(out=bias_sb[0], in_=bias_0[:, :])
            nc.sync.dma_start(out=bias_sb[1], in_=bias_1[:, :])
            bns_sb = [consts.tile([F, 1], FP, name=f"bnssb{i}") for i in range(2)]
            nc.sync.dma_start(out=bns_sb[0], in_=bn0s[:, :])
            nc.sync.dma_start(out=bns_sb[1], in_=bn1s[:, :])
            bnh_sb = [consts.tile([F, 1], FP, name=f"bnhsb{i}") for i in range(2)]
            nc.sync.dma_start(out=bnh_sb[0], in_=bn0h[:, :])
            nc.sync.dma_start(out=bnh_sb[1], in_=bn1h[:, :])
            attnk_sb = consts.tile([F, 1], FP)
            nc.sync.dma_start(out=attnk_sb, in_=attn_k[:, :])
            densew_sb = consts.tile([F, D], FP)
            nc.sync.dma_start(out=densew_sb, in_=dense_w[:, :])
            densebc_sb = consts.tile([N, DC], FP)
            nc.sync.dma_start(out=densebc_sb, in_=dense_bc[:, :])
            wf_sb = consts.tile([N, DC], FP)
            nc.sync.dma_start(out=wf_sb, in_=wf[:, :])
            cf_sb = consts.tile([1, 1], FP)
            nc.sync.dma_start(out=cf_sb, in_=cf[:, :])

            ident = consts.tile([N, N], FP)
            make_identity(nc, ident)
            ones_f = consts.tile([1, F], FP)
            nc.vector.memset(ones_f, 1.0)
            ones_n = consts.tile([N, 1], FP)
            nc.vector.memset(ones_n, 1.0)

            # ---------------- transposes ----------------
            # aT[j, i] = a[i, j]
            ps_a = ps_tr.tile([N, N], FP, tag="tr")
            nc.tensor.transpose(ps_a, a_sb, ident)
            a_T = big.tile([N, N], FP)
            nc.vector.tensor_copy(a_T, ps_a)

            # xT[f, i] = x[i, f]
            ps_x = ps_tr.tile([F, N], FP, tag="tr")
            nc.tensor.transpose(ps_x, x_sb, ident)
            x_T = consts.tile([F, N], FP)
            nc.vector.tensor_copy(x_T, ps_x)

            # AE_T[j, s*N + i] = a[i, j] * e[i, j, s]
            e_v = e_sb.rearrange("p (j s) -> p s j", s=S)
            ae_T = big.tile([N, S * N], FP)
            for s in range(S):
                ps_e = ps_tr.tile([N, N], FP, tag="tr", name=f"ps_e{s}")
                nc.tensor.transpose(ps_e, e_v[:, s, :], ident)
                nc.vector.tensor_mul(ae_T[:, s * N:(s + 1) * N], ps_e, a_T)

            # ---------------- ECC layers ----------------
            def ecc_layer(h_T, li):
                # P16[j, s*F + o] = sum_f h[j, f] * W16[f, s*F + o]
                ps16 = ps_p16p.tile([N, SB * F], FP, tag="p16", name=f"ps16_{li}")
                nc.tensor.matmul(ps16[:, 0:512], lhsT=h_T, rhs=w16_sb[li][:, 0:512],
                                 start=True, stop=True)
                nc.tensor.matmul(ps16[:, 512:SB * F], lhsT=h_T,
                                 rhs=w16_sb[li][:, 512:SB * F], start=True, stop=True)
                p16 = work.tile([N, SB * F], FP, tag="p16sb", name=f"p16_{li}")
                nc.vector.tensor_copy(p16, ps16)

                # outT[o, i] = sum_s sum_j P16[j, s, o] * AE_T[j, s*N + i]
                #            + sum_j P16[j, bias, o] * aT[j, i]
                #            + sum_f root[f, o] * hT[f, i]
                ps_o = ps_misc.tile([F, N], FP, tag="acc", name=f"ps_o{li}")
                for s in range(S):
                    nc.tensor.matmul(ps_o, lhsT=p16[:, s * F:(s + 1) * F],
                                     rhs=ae_T[:, s * N:(s + 1) * N],
                                     start=(s == 0), stop=False)
                nc.tensor.matmul(ps_o, lhsT=p16[:, S * F:SB * F], rhs=a_T,
                                 start=False, stop=False)
                nc.tensor.matmul(ps_o, lhsT=root_sb[li], rhs=h_T,
                                 start=False, stop=True)

                # tanh(out + bias), then folded batchnorm affine
                ht = work.tile([F, N], FP, tag="h", name=f"h{li}")
                nc.scalar.activation(ht, ps_o, AF.Tanh, bias=bias_sb[li], scale=1.0)
                nc.vector.tensor_scalar(ht, ht, bns_sb[li], bnh_sb[li],
                                        AL.mult, AL.add)
                return ht

            h1_T = ecc_layer(x_T, 0)
            h2_T = ecc_layer(h1_T, 1)

            # ---------------- global attention sum pool ----------------
            ps_l = ps_misc.tile([1, N], FP, tag="small")
            nc.tensor.matmul(ps_l, lhsT=attnk_sb, rhs=h2_T, start=True, stop=True)
            neg_mx = work.tile([1, 1], FP, tag="negmx")
            nc.vector.tensor_reduce(neg_mx, ps_l, axis=mybir.AxisListType.X,
                                    op=AL.max, negate=True)
            exps = work.tile([1, N], FP, tag="exps")
            nc.scalar.activation(exps, ps_l, AF.Exp, bias=neg_mx, scale=1.0)
            ssum = work.tile([1, 1], FP, tag="ssum")
            nc.vector.tensor_reduce(ssum, exps, axis=mybir.AxisListType.X, op=AL.add)
            rsum = work.tile([1, 1], FP, tag="rsum")
            nc.vector.reciprocal(rsum, ssum)
            attn = work.tile([1, N], FP, tag="attn")
            nc.vector.tensor_scalar_mul(attn, exps, rsum)

            # replicate attn across F partitions, then g[f] = sum_i attn[i]*h2[f,i]
            ps_rep = ps_misc.tile([F, N], FP, tag="acc")
            nc.tensor.matmul(ps_rep, lhsT=ones_f, rhs=attn, start=True, stop=True)
            junk = work.tile([F, N], FP, tag="junk")
            g_T = work.tile([F, 1], FP, tag="gT")
            nc.vector.tensor_mul(junk, h2_T, ps_rep)
            nc.vector.tensor_reduce(g_T, junk, axis=mybir.AxisListType.X, op=AL.add)

            # ---------------- dense head ----------------
            # z[c*N + p] = sum_f dense_w[f, c*N+p] * g[f]   (column-chunk layout)
            ps_z = ps_misc.tile([N, DC], FP, tag="small")
            for c in range(DC):
                nc.tensor.matmul(ps_z[:, c:c + 1], lhsT=densew_sb[:, c * N:(c + 1) * N],
                                 rhs=g_T, start=True, stop=True)
            zb = work.tile([N, DC], FP, tag="zb")
            nc.vector.tensor_add(zb, ps_z, densebc_sb)
            zl = work.tile([N, DC], FP, tag="zl")
            nc.vector.scalar_tensor_tensor(zl, in0=zb, scalar=ALPHA, in1=zb,
                                           op0=AL.mult, op1=AL.max)

            # y = sum(zl * wf) + cf   (bnf affine + mle_w folded into wf/cf)
            junk2 = work.tile([N, DC], FP, tag="junk2")
            ypart = work.tile([N, 1], FP, tag="ypart")
            nc.vector.tensor_mul(junk2, zl, wf_sb)
            nc.vector.tensor_reduce(ypart, junk2, axis=mybir.AxisListType.X, op=AL.add)
            ps_y = ps_misc.tile([1, 1], FP, tag="acc", name="ps_y")
            nc.tensor.matmul(ps_y, lhsT=ypart, rhs=ones_n, start=True, stop=True)
            y_sb = work.tile([1, 1], FP, tag="ysb")
            nc.scalar.activation(y_sb, ps_y, AF.Identity, bias=cf_sb, scale=1.0)
            nc.sync.dma_start(out=y_out[:, :], in_=y_sb)

    nc.compile()
    return nc


def _get_nc():
    global _NC_CACHE
    if _NC_CACHE is None:
        _NC_CACHE = _build_nc()
    return _NC_CACHE


def _host_prep(inputs):
    """Fold emb_w / kn_b / batchnorms / mle into compact weight tensors."""
    f32 = np.float32
    shared = {}
    for li in range(2):
        kn_w = np.asarray(inputs[f"kn_w{li}"], f32)          # [8, Fo*F]
        kn_b = np.asarray(inputs[f"kn_b{li}"], f32)          # [Fo*F]
        emb_w = np.asarray(inputs["emb_w"], f32)             # [S, 8]
        kn = kn_w.reshape(8, F, F)                            # [k, o, f]
        w16 = np.einsum("sk,kof->fso", emb_w, kn)             # [f, s, o]
        bb = kn_b.reshape(F, F).T                             # [f, o]
        w16cat = np.concatenate([w16.reshape(F, S * F), bb], axis=1)  # [F, 17*F]
        shared[f"w16_{li}"] = np.ascontiguousarray(w16cat)
        shared[f"root_{li}"] = np.ascontiguousarray(np.asarray(inputs[f"root{li}"], f32))
        shared[f"bias_{li}"] = np.ascontiguousarray(
            np.asarray(inputs[f"bias{li}"], f32).reshape(F, 1))
        g = np.asarray(inputs[f"bn_g{li}"], f32)
        bb_ = np.asarray(inputs[f"bn_b{li}"], f32)
        m = np.asarray(inputs[f"bn_m{li}"], f32)
        v = np.asarray(inputs[f"bn_v{li}"], f32)
        sc = g / np.sqrt(v + EPS)
        shared[f"bn{li}s"] = np.ascontiguousarray(sc.reshape(F, 1))
        shared[f"bn{li}h"] = np.ascontiguousarray((bb_ - m * sc).reshape(F, 1))

    shared["attn_k"] = np.ascontiguousarray(np.asarray(inputs["attn_k"], f32))
    shared["dense_w"] = np.ascontiguousarray(np.asarray(inputs["dense_w"], f32))
    shared["dense_bc"] = np.ascontiguousarray(
        np.asarray(inputs["dense_b"], f32).reshape(DC, N).T)
    bnfs = np.asarray(inputs["bnf_g"], f32) / np.sqrt(np.asarray(inputs["bnf_v"], f32) + EPS)
    mw = np.asarray(inputs["mle_w"], f32)[:, 0]
    shared["wf"] = np.ascontiguousarray((bnfs * mw).reshape(DC, N).T)
    cf = np.sum((np.asarray(inputs["bnf_b"], f32)
                 - np.asarray(inputs["bnf_m"], f32) * bnfs) * mw) + np.asarray(inputs["mle_b"], f32)[0]
    shared["cf"] = np.full((1, 1), cf, f32)
    return shared


def _make_in_maps(inputs):
    shared = _host_prep(inputs)
    e = np.asarray(inputs["e"], np.float32)
    a = np.asarray(inputs["a"], np.float32)
    x = np.asarray(inputs["x"], np.float32)
    in_maps = []
    for c in range(NCORES):
        b = c % B
        m = dict(shared)
        m["e_b"] = np.ascontiguousarray(e[b].reshape(N, N * S))
        m["a_b"] = np.ascontiguousarray(a[b])
        m["x_b"] = np.ascontiguousarray(x[b])
        in_maps.append(m)
    return in_maps


def _run(inputs, trace=False, trace_cores=None):
    nc = _get_nc()
    in_maps = _make_in_maps(inputs)
    res = run_bass_kernel_spmd(nc, in_maps, list(range(NCORES)),
                               trace=trace, trace_cores=trace_cores)
    out = np.empty((B + 1, 1), np.float32)
    for b in range(B):
        out[b, 0] = res.results[b]["y"][0, 0]
    out[B, 0] = np.asarray(inputs["sigma"], np.float32)[0, 0]
    return out, res


def kernel(**inputs):
    out, _ = _run(inputs)
    return out


# revision 9
# speedup vs baseline: 1.2999x; 1.2999x over previous
"""GCNN (spektral ECCConv x2 + attn-sum-pool + dense head) Trainium2 kernel.

Strategy
--------
The ECC "kernel network" is a single linear layer, so the per-edge kernel
tensor [B,N,N,Fo*F] never needs to exist.  Folding emb_w into kn_w gives
W16[s,o,f] (s over the 16 raw edge channels + 1 bias channel) and

    out[b,i,o] = sum_{j,s} a[b,i,j]*e[b,i,j,s] * P16[b,j,s,o] + (h @ root)[b,i,o]
    P16[b,j,s,o] = sum_f W16[s,o,f] * h[b,j,f]

Everything is kept feature-major ("transposed", [F, node]) on chip:
  - aT and the 16 e-slices are transposed once on the TensorEngine
    (AE_T[j, s*128+i] = a[b,i,j]*e[b,i,j,s], reused by both layers)
  - per layer: one [32,128]x[32,544] matmul produces P16, then 17
    accumulating [128,32]x[128,128] matmuls + one root matmul produce
    the layer output directly in [Fo, node] layout.
All TensorEngine operands are bf16 (fp32 runs 2-pass LOW_HIGH on the PE,
~4x slower); PSUM accumulation stays fp32, as do softmax and the scalar
tail, keeping overall rel-err ~1e-3.
Distribution: data-parallel over batch (B=4) on cores 0-3; cores 4-7
process duplicate data and are ignored.

Shapes are hardcoded for B=4, N=128, F0=32, S=16, E8=8, C=[32,32].
"""

import numpy as np
import ml_dtypes

import concourse.bass as bass
import concourse.mybir as mybir
import concourse.tile as tile
from concourse import bacc
from concourse.bass_utils import run_bass_kernel_spmd
from concourse.masks import make_identity

FP = mybir.dt.float32
BF = mybir.dt.float16
AL = mybir.AluOpType
AF = mybir.ActivationFunctionType

N = 128       # nodes
F = 32        # feature dim of both ECC layers (F0 = C0 = C1 = 32)
S = 16        # raw edge feature dim
SB = S + 1    # + kernel-network bias channel
D = 1024      # dense head width
DC = D // N   # dense head column chunks (8)
EPS = 1e-3    # Keras BatchNormalization epsilon
ALPHA = 0.01  # LeakyReLU
B = 4
NCORES = 8

_NC_CACHE = None


def _build_nc():
    nc = bacc.Bacc("TRN2", target_bir_lowering=False, debug=False)

    e_b = nc.dram_tensor("e_b", [N, N * S], FP, kind="ExternalInput")
    a_b = nc.dram_tensor("a_b", [N, N], FP, kind="ExternalInput")
    x_b = nc.dram_tensor("x_b", [N, F], FP, kind="ExternalInput")
    w16_0 = nc.dram_tensor("w16_0", [F, SB * F], BF, kind="ExternalInput")
    w16_1 = nc.dram_tensor("w16_1", [F, SB * F], BF, kind="ExternalInput")
    root_0 = nc.dram_tensor("root_0", [F, F], BF, kind="ExternalInput")
    root_1 = nc.dram_tensor("root_1", [F, F], BF, kind="ExternalInput")
    bias_0 = nc.dram_tensor("bias_0", [F, 1], FP, kind="ExternalInput")
    bias_1 = nc.dram_tensor("bias_1", [F, 1], FP, kind="ExternalInput")
    bn0s = nc.dram_tensor("bn0s", [F, 1], FP, kind="ExternalInput")
    bn0h = nc.dram_tensor("bn0h", [F, 1], FP, kind="ExternalInput")
    bn1s = nc.dram_tensor("bn1s", [F, 1], FP, kind="ExternalInput")
    bn1h = nc.dram_tensor("bn1h", [F, 1], FP, kind="ExternalInput")
    attn_k = nc.dram_tensor("attn_k", [F, 1], BF, kind="ExternalInput")
    dense_w = nc.dram_tensor("dense_w", [F, D], BF, kind="ExternalInput")
    dense_bc = nc.dram_tensor("dense_bc", [N, DC], FP, kind="ExternalInput")
    wf = nc.dram_tensor("wf", [N, DC], FP, kind="ExternalInput")
    cf = nc.dram_tensor("cf", [1, 1], FP, kind="ExternalInput")
    y_out = nc.dram_tensor("y", [1, 1], FP, kind="ExternalOutput")

    with tile.TileContext(nc) as tc:
        with (
            tc.tile_pool(name="big", bufs=1) as big,
            tc.tile_pool(name="consts", bufs=1) as consts,
            tc.tile_pool(name="work", bufs=2) as work,
            tc.tile_pool(name="ps_tr", bufs=2, space="PSUM") as ps_tr,
            tc.tile_pool(name="ps_p16", bufs=1, space="PSUM") as ps_p16p,
            tc.tile_pool(name="ps_misc", bufs=1, space="PSUM") as ps_misc,
        ):
            # ---------------- loads ----------------
            a_sb = consts.tile([N, N], FP)
            nc.sync.dma_start(out=a_sb, in_=a_b[:, :])
            x_sb = consts.tile([N, F], BF)
            nc.gpsimd.dma_start(out=x_sb, in_=x_b[:, :])      # SWDGE casts f32->bf16
            e_sb = big.tile([N, N * S], BF)
            nc.gpsimd.dma_start(out=e_sb, in_=e_b[:, :])      # SWDGE casts f32->bf16

            w16_sb = [consts.tile([F, SB * F], BF, name=f"w16sb{i}") for i in range(2)]
            nc.sync.dma_start(out=w16_sb[0], in_=w16_0[:, :])
            nc.sync.dma_start(out=w16_sb[1], in_=w16_1[:, :])
            root_sb = [consts.tile([F, F], BF, name=f"rootsb{i}") for i in range(2)]
            nc.sync.dma_start(out=root_sb[0], in_=root_0[:, :])
            nc.sync.dma_start(out=root_sb[1], in_=root_1[:, :])
            bias_sb = [consts.tile([F, 1], FP, name=f"biassb{i}") for i in range(2)]
            nc.sync.dma_start(out=bias_sb[0], in_=bias_0[:, :])
            nc.sync.dma_start(out=bias_sb[1], in_=bias_1[:, :])
            bns_sb = [consts.tile([F, 1], FP, name=f"bnssb{i}") for i in range(2)]
            nc.sync.dma_start(out=bns_sb[0], in_=bn0s[:, :])
            nc.sync.dma_start(out=bns_sb[1], in_=bn1s[:, :])
            bnh_sb = [consts.tile([F, 1], FP, name=f"bnhsb{i}") for i in range(2)]
            nc.sync.dma_start(out=bnh_sb[0], in_=bn0h[:, :])
            nc.sync.dma_start(out=bnh_sb[1], in_=bn1h[:, :])
            attnk_sb = consts.tile([F, 1], BF)
            nc.sync.dma_start(out=attnk_sb, in_=attn_k[:, :])
            densew_sb = consts.tile([F, D], BF)
            nc.sync.dma_start(out=densew_sb, in_=dense_w[:, :])
            densebc_sb = consts.tile([N, DC], FP)
            nc.sync.dma_start(out=densebc_sb, in_=dense_bc[:, :])
            wf_sb = consts.tile([N, DC], FP)
            nc.sync.dma_start(out=wf_sb, in_=wf[:, :])
            cf_sb = consts.tile([1, 1], FP)
            nc.sync.dma_start(out=cf_sb, in_=cf[:, :])

            ident = consts.tile([N, N], FP)
            make_identity(nc, ident)
            ident_bf = consts.tile([N, N], BF)
            nc.vector.tensor_copy(ident_bf, ident)
            ones_f = consts.tile([1, F], FP)
            nc.vector.memset(ones_f, 1.0)
            ones_n = consts.tile([N, 1], FP)
            nc.vector.memset(ones_n, 1.0)

            # ---------------- transposes ----------------
            # aT[j, i] = a[i, j]  (fp32 for the DVE muls; bf16 copy for matmuls)
            ps_a = ps_tr.tile([N, N], FP, tag="trf", bufs=1)
            nc.tensor.transpose(ps_a, a_sb, ident)
            a_Tb = big.tile([N, N], BF)
            nc.vector.tensor_copy(a_Tb, ps_a)

            # xT[f, i] = x[i, f]
            ps_x = ps_tr.tile([F, N], BF, tag="tr")
            nc.tensor.transpose(ps_x, x_sb, ident_bf)
            x_T = consts.tile([F, N], BF)
            nc.vector.tensor_copy(x_T, ps_x)

            # AE_T[j, s*N + i] = a[i, j] * e[i, j, s]
            e_v = e_sb.rearrange("p (j s) -> p s j", s=S)
            ae_T = big.tile([N, S * N], BF)
            for s in range(S):
                ps_e = ps_tr.tile([N, N], BF, tag="tr", name=f"ps_e{s}")
                nc.tensor.transpose(ps_e, e_v[:, s, :], ident_bf)
                nc.vector.tensor_mul(ae_T[:, s * N:(s + 1) * N], ps_e, a_Tb)

            # ---------------- ECC layers ----------------
            def ecc_layer(h_T, li):
                # P16[j, s*F + o] = sum_f h[j, f] * W16[f, s*F + o]
                ps16 = ps_p16p.tile([N, SB * F], FP, tag="p16", name=f"ps16_{li}")
                nc.tensor.matmul(ps16[:, 0:512], lhsT=h_T, rhs=w16_sb[li][:, 0:512],
                                 start=True, stop=True)
                nc.tensor.matmul(ps16[:, 512:SB * F], lhsT=h_T,
                                 rhs=w16_sb[li][:, 512:SB * F], start=True, stop=True)
                p16 = work.tile([N, SB * F], BF, tag="p16sb", name=f"p16_{li}")
                nc.vector.tensor_copy(p16, ps16)

                # outT[o, i] = sum_s sum_j P16[j, s, o] * AE_T[j, s*N + i]
                #            + sum_j P16[j, bias, o] * aT[j, i]
                #            + sum_f root[f, o] * hT[f, i]
                ps_o = ps_misc.tile([F, N], FP, tag="acc", name=f"ps_o{li}")
                for s in range(S):
                    nc.tensor.matmul(ps_o, lhsT=p16[:, s * F:(s + 1) * F],
                                     rhs=ae_T[:, s * N:(s + 1) * N],
                                     start=(s == 0), stop=False)
                nc.tensor.matmul(ps_o, lhsT=p16[:, S * F:SB * F], rhs=a_Tb,
                                 start=False, stop=False)
                nc.tensor.matmul(ps_o, lhsT=root_sb[li], rhs=h_T,
                                 start=False, stop=True)

                # tanh(out + bias), then folded batchnorm affine (casts to bf16)
                hp = work.tile([F, N], FP, tag="hpre", name=f"hp{li}")
                nc.scalar.activation(hp, ps_o, AF.Tanh, bias=bias_sb[li], scale=1.0)
                ht = work.tile([F, N], BF, tag="h", name=f"h{li}")
                nc.vector.tensor_scalar(ht, hp, bns_sb[li], bnh_sb[li],
                                        AL.mult, AL.add)
                return ht

            h1_T = ecc_layer(x_T, 0)
            h2_T = ecc_layer(h1_T, 1)

            # ---------------- global attention sum pool ----------------
            ps_l = ps_misc.tile([1, N], FP, tag="small")
            nc.tensor.matmul(ps_l, lhsT=attnk_sb, rhs=h2_T, start=True, stop=True)
            neg_mx = work.tile([1, 1], FP, tag="negmx")
            nc.vector.tensor_reduce(neg_mx, ps_l, axis=mybir.AxisListType.X,
                                    op=AL.max, negate=True)
            exps = work.tile([1, N], FP, tag="exps")
            nc.scalar.activation(exps, ps_l, AF.Exp, bias=neg_mx, scale=1.0)
            ssum = work.tile([1, 1], FP, tag="ssum")
            nc.vector.tensor_reduce(ssum, exps, axis=mybir.AxisListType.X, op=AL.add)
            rsum = work.tile([1, 1], FP, tag="rsum")
            nc.vector.reciprocal(rsum, ssum)
            attn = work.tile([1, N], FP, tag="attn")
            nc.vector.tensor_scalar_mul(attn, exps, rsum)

            # replicate attn across F partitions, then g[f] = sum_i attn[i]*h2[f,i]
            ps_rep = ps_misc.tile([F, N], FP, tag="acc", name="ps_rep")
            nc.tensor.matmul(ps_rep, lhsT=ones_f, rhs=attn, start=True, stop=True)
            rep_sb = work.tile([F, N], BF, tag="repsb")
            nc.vector.tensor_copy(rep_sb, ps_rep)
            junk = work.tile([F, N], FP, tag="junk")
            nc.vector.tensor_mul(junk, h2_T, rep_sb)
            g_T = work.tile([F, 1], FP, tag="gT")
            nc.vector.tensor_reduce(g_T, junk, axis=mybir.AxisListType.X, op=AL.add)
            g_Tb = work.tile([F, 1], BF, tag="gTb")
            nc.vector.tensor_copy(g_Tb, g_T)

            # ---------------- dense head ----------------
            # z[c*N + p] = sum_f dense_w[f, c*N+p] * g[f]   (column-chunk layout)
            ps_z = ps_misc.tile([N, DC], FP, tag="small", name="ps_z")
            for c in range(DC):
                nc.tensor.matmul(ps_z[:, c:c + 1], lhsT=densew_sb[:, c * N:(c + 1) * N],
                                 rhs=g_Tb, start=True, stop=True)
            zb = work.tile([N, DC], FP, tag="zb")
            nc.vector.tensor_add(zb, ps_z, densebc_sb)
            zl = work.tile([N, DC], FP, tag="zl")
            nc.vector.scalar_tensor_tensor(zl, in0=zb, scalar=ALPHA, in1=zb,
                                           op0=AL.mult, op1=AL.max)

            # y = sum(zl * wf) + cf   (bnf affine + mle_w folded into wf/cf)
            junk2 = work.tile([N, DC], FP, tag="junk2")
            nc.vector.tensor_mul(junk2, zl, wf_sb)
            ypart = work.tile([N, 1], FP, tag="ypart")
            nc.vector.tensor_reduce(ypart, junk2, axis=mybir.AxisListType.X, op=AL.add)
            ps_y = ps_misc.tile([1, 1], FP, tag="acc", name="ps_y")
            nc.tensor.matmul(ps_y, lhsT=ypart, rhs=ones_n, start=True, stop=True)
            y_sb = work.tile([1, 1], FP, tag="ysb")
            nc.scalar.activation(y_sb, ps_y, AF.Identity, bias=cf_sb, scale=1.0)
            nc.sync.dma_start(out=y_out[:, :], in_=y_sb)

    nc.compile()
    return nc


def _get_nc():
    global _NC_CACHE
    if _NC_CACHE is None:
        _NC_CACHE = _build_nc()
    return _NC_CACHE


def _host_prep(inputs):
    """Fold emb_w / kn_b / batchnorms / mle into compact weight tensors."""
    f32 = np.float32
    bf16 = np.float16
    shared = {}
    for li in range(2):
        kn_w = np.asarray(inputs[f"kn_w{li}"], f32)          # [8, Fo*F]
        kn_b = np.asarray(inputs[f"kn_b{li}"], f32)          # [Fo*F]
        emb_w = np.asarray(inputs["emb_w"], f32)             # [S, 8]
        kn = kn_w.reshape(8, F, F)                            # [k, o, f]
        w16 = np.einsum("sk,kof->fso", emb_w, kn)             # [f, s, o]
        bb = kn_b.reshape(F, F).T                             # [f, o]
        w16cat = np.concatenate([w16.reshape(F, S * F), bb], axis=1)  # [F, 17*F]
        shared[f"w16_{li}"] = np.ascontiguousarray(w16cat.astype(bf16))
        shared[f"root_{li}"] = np.ascontiguousarray(
            np.asarray(inputs[f"root{li}"], f32).astype(bf16))
        shared[f"bias_{li}"] = np.ascontiguousarray(
            np.asarray(inputs[f"bias{li}"], f32).reshape(F, 1))
        g = np.asarray(inputs[f"bn_g{li}"], f32)
        bb_ = np.asarray(inputs[f"bn_b{li}"], f32)
        m = np.asarray(inputs[f"bn_m{li}"], f32)
        v = np.asarray(inputs[f"bn_v{li}"], f32)
        sc = g / np.sqrt(v + EPS)
        shared[f"bn{li}s"] = np.ascontiguousarray(sc.reshape(F, 1))
        shared[f"bn{li}h"] = np.ascontiguousarray((bb_ - m * sc).reshape(F, 1))

    shared["attn_k"] = np.ascontiguousarray(
        np.asarray(inputs["attn_k"], f32).astype(bf16))
    shared["dense_w"] = np.ascontiguousarray(
        np.asarray(inputs["dense_w"], f32).astype(bf16))
    shared["dense_bc"] = np.ascontiguousarray(
        np.asarray(inputs["dense_b"], f32).reshape(DC, N).T)
    bnfs = np.asarray(inputs["bnf_g"], f32) / np.sqrt(np.asarray(inputs["bnf_v"], f32) + EPS)
    mw = np.asarray(inputs["mle_w"], f32)[:, 0]
    shared["wf"] = np.ascontiguousarray((bnfs * mw).reshape(DC, N).T)
    cf = np.sum((np.asarray(inputs["bnf_b"], f32)
                 - np.asarray(inputs["bnf_m"], f32) * bnfs) * mw) + np.asarray(inputs["mle_b"], f32)[0]
    shared["cf"] = np.full((1, 1), cf, f32)
    return shared


def _make_in_maps(inputs):
    shared = _host_prep(inputs)
    e = np.asarray(inputs["e"], np.float32)
    a = np.asarray(inputs["a"], np.float32)
    x = np.asarray(inputs["x"], np.float32)
    in_maps = []
    for c in range(NCORES):
        b = c % B
        m = dict(shared)
        m["e_b"] = np.ascontiguousarray(e[b].reshape(N, N * S))
        m["a_b"] = np.ascontiguousarray(a[b])
        m["x_b"] = np.ascontiguousarray(x[b])
        in_maps.append(m)
    return in_maps


def _run(inputs, trace=False, trace_cores=None):
    nc = _get_nc()
    in_maps = _make_in_maps(inputs)
    res = run_bass_kernel_spmd(nc, in_maps, list(range(NCORES)),
                               trace=trace, trace_cores=trace_cores)
    out = np.empty((B + 1, 1), np.float32)
    for b in range(B):
        out[b, 0] = res.results[b]["y"][0, 0]
    out[B, 0] = np.asarray(inputs["sigma"], np.float32)[0, 0]
    return out, res


def kernel(**inputs):
    out, _ = _run(inputs)
    return out
